# revision 11
# baseline (speedup 1.0000x reference)
"""Bahdanau attention TRN2 kernel (8 NeuronCores, data-parallel over batch).

Problem: B=32, S=4096, ENC=DEC=ATT=512.
  enc_score = enc @ W_enc^T + W_enc_b            [B,S,A]
  dec_score = dec @ W_dec^T + W_dec_b            [B,A]
  align  = tanh(enc_score + dec_score + bias)    [B,S,A]
  scores = align @ V + V_b                       [B,S]
  attn   = softmax(scores, -1)                   [B,S]
  context= attn @ enc                            [B,E]

Host-side prep (cheap, O(weights + one pass over enc)):
  - db = dec@W_dec^T + W_dec_b + bias + W_enc_b folds every per-(b,a)
    additive term into one [B,A] tensor. V_b is dropped entirely: softmax is
    shift invariant, so it cannot affect either output.
  - enc is shipped twice in bf16: natural layout (for the context matmuls)
    and pre-transposed 128x128 blocks (stationary operands for the score
    matmul) - no on-device transposes at all.

Device per example (4 per core):
  - score matmul: stationary = encT block [e=128, s=128], moving = W_encT
    chunk [e=128, a=512], PSUM f32 accumulation over 4 e-chunks; the db row
    is added with a K=1 ones-matmul into the same PSUM group.
  - tanh on ACT straight out of PSUM (FD=1024 per instruction).
  - scores: DVE multiply by V (bf16 2x mode), then free-dim reduce split
    between DVE tensor_reduce and ACT activation-accumulate.
  - exp on ACT ([128,32] per example; scores are bounded by sum|V| ~ 5.7 so
    no max-subtraction is needed). Unnormalized exp goes straight to DRAM.
  - context: 32 M=1 PE matmuls (exp_bf16 stationary, natural enc moving)
    accumulating [1,512] in PSUM.
  - Softmax normalization of both outputs happens on host, exactly.
"""

import sys

sys.path.insert(0, "/opt/trn_rl_repo")

import numpy as np
import ml_dtypes

import concourse.bass as bass
import concourse.mybir as mybir
from concourse.tile import TileContext

B, S, ENC, ATT = 32, 4096, 512, 512
N_CORES = 8
BPC = B // N_CORES  # examples per core
NT = S // 128       # s-tiles per example (32)
NCH = 4             # encT stream chunks per example
TPC = NT // NCH     # s-tiles per chunk (8)
BF16 = mybir.dt.bfloat16
F32 = mybir.dt.float32

# ---------------------------------------------------------------------------
# Workaround: this container's walrus accepts at most one sync-wait per
# instruction (two for EventSemaphore); Tile emits several. Split the extras
# onto single-wait NOPs on the same engine right before the instruction.
_PATCHED = False


def _patch_drain():
    global _PATCHED
    if _PATCHED:
        return
    from bass_rust import ScopedClock

    def _drain_and_barrier(self, tick_clock, wait_clock):
        probe = self.nc.sync.nop(nofuse=True, hint="drain_wait_hoist")
        wait_clock.add_sem_waits(
            probe.ins, ScopedClock({None: tick_clock.global_clock})
        )
        si = probe.ins.sync_info
        waits = list(si.on_wait or []) if si is not None else []
        if len(waits) > 1:
            si.on_wait = waits[:1]
            for w in waits[1:]:
                n2 = self.nc.sync.nop(nofuse=True, hint="drain_wait_hoist")
                n2.ins.sync_info = mybir.SyncInfo(on_wait=[w], on_update=[])
        self.nc.sync.drain()
        self.nc.all_engine_barrier()
        assert self.sems is not None
        popped = self.nc._tile_sem_poison_stack.pop()
        assert popped is self._sem_poison
        self.nc.clear_and_free_semaphores(list(self.sems.allocated().values()))
        self.nc.all_engine_barrier()

    TileContext._drain_and_barrier = _drain_and_barrier
    _PATCHED = True


def _split_sync_waits(nc):
    ctr = [0]

    def mknop(engine, wait):
        ctr[0] += 1
        n = mybir.InstNoOp(name=f"I-wsplit-{ctr[0]}", ins=[], outs=[])
        n.engine = engine
        n.sync_info = mybir.SyncInfo(on_wait=[wait], on_update=[])
        return n

    for fn in nc.m.functions:
        for bb in fn.blocks:
            out = []
            changed = False
            for inst in bb.instructions:
                si = inst.sync_info
                waits = list(si.on_wait) if (si and si.on_wait) else []
                cap = 2 if isinstance(inst, mybir.InstEventSemaphore) else 1
                if len(waits) > cap:
                    changed = True
                    for w in waits[: len(waits) - cap]:
                        out.append(mknop(inst.engine, w))
                    si.on_wait = waits[len(waits) - cap :]
                out.append(inst)
            if changed:
                bb.instructions = out


# ---------------------------------------------------------------------------
def build_nc():
    _patch_drain()
    nc = bass.Bass()
    enc_d = nc.dram_tensor("enc", [BPC, S, ENC], BF16, kind="ExternalInput")
    encT_d = nc.dram_tensor(
        "encT", [BPC, NCH, 4, 128, TPC, 128], BF16, kind="ExternalInput"
    )
    wt_d = nc.dram_tensor("wt", [ENC, ATT], BF16, kind="ExternalInput")
    db_d = nc.dram_tensor("db", [1, BPC, ATT], BF16, kind="ExternalInput")
    vb_d = nc.dram_tensor("vb", [128, ATT], BF16, kind="ExternalInput")
    ones_d = nc.dram_tensor("ones", [1, 128], BF16, kind="ExternalInput")
    ctx_d = nc.dram_tensor("ctx", [BPC, ENC], F32, kind="ExternalOutput")
    att_d = nc.dram_tensor("att", [BPC, S], F32, kind="ExternalOutput")

    Tanh = mybir.ActivationFunctionType.Tanh
    Exp = mybir.ActivationFunctionType.Exp
    Ident = mybir.ActivationFunctionType.Identity

    with TileContext(nc) as tc:
        import contextlib

        with contextlib.ExitStack() as ctx:
            consts = ctx.enter_context(tc.tile_pool(name="consts", bufs=1))
            natp = ctx.enter_context(tc.tile_pool(name="nat", bufs=1))
            etp = ctx.enter_context(tc.tile_pool(name="et", bufs=3))
            thp = ctx.enter_context(tc.tile_pool(name="th", bufs=3))
            scrp = ctx.enter_context(tc.tile_pool(name="scr", bufs=3))
            dmpp = ctx.enter_context(tc.tile_pool(name="dmp", bufs=3))
            scp = ctx.enter_context(tc.tile_pool(name="sc", bufs=1))
            smallp = ctx.enter_context(tc.tile_pool(name="small", bufs=2))
            mmps = ctx.enter_context(
                tc.tile_pool(name="mmps", bufs=3, space="PSUM")
            )
            ctxps = ctx.enter_context(
                tc.tile_pool(name="ctxps", bufs=2, space="PSUM")
            )

            wt_sb = consts.tile([128, 4, ATT], BF16)
            nc.sync.dma_start(
                out=wt_sb, in_=wt_d.ap().rearrange("(j p) a -> p j a", p=128)
            )
            vb_sb = consts.tile([128, ATT], BF16)
            nc.sync.dma_start(out=vb_sb, in_=vb_d.ap())
            db_sb = consts.tile([1, BPC, ATT], BF16)
            nc.sync.dma_start(out=db_sb, in_=db_d.ap())
            ones_sb = consts.tile([1, 128], BF16)
            nc.sync.dma_start(out=ones_sb, in_=ones_d.ap())

            def emit_scores(b):
                nat_b = natp.tile([128, NT, ENC], BF16, tag=f"nat{b % 2}")
                nc.sync.dma_start(
                    out=nat_b,
                    in_=enc_d.ap()[b].rearrange("(t p) e -> p t e", p=128),
                )
                sc_sb = scp.tile([128, NT], F32, tag=f"sc{b % 2}")
                for ch in range(NCH):
                    etc = etp.tile([128, 4, TPC, 128], BF16, tag="etc")
                    nc.sync.dma_start(
                        out=etc,
                        in_=encT_d.ap()[b][ch].rearrange(
                            "j p k s -> p j k s"
                        ),
                    )
                    for pair in range(TPC // 2):
                        ps = mmps.tile([128, 2, ATT], F32, tag="mm")
                        for k2 in range(2):
                            k = pair * 2 + k2
                            for j in range(4):
                                nc.tensor.matmul(
                                    ps[:, k2, :],
                                    lhsT=etc[:, j, k, :],
                                    rhs=wt_sb[:, j, :],
                                    start=(j == 0),
                                    stop=False,
                                )
                            nc.tensor.matmul(
                                ps[:, k2, :],
                                lhsT=ones_sb,
                                rhs=db_sb[:, b, :],
                                start=False,
                                stop=True,
                            )
                        th = thp.tile([128, 2, ATT], BF16, tag="th")
                        nc.scalar.activation(th, ps, Tanh)
                        for k2 in range(2):
                            t = ch * TPC + pair * 2 + k2
                            scr = scrp.tile([128, ATT], BF16, tag="scr")
                            nc.vector.tensor_mul(scr, th[:, k2, :], vb_sb)
                            if t % 3 != 2:
                                nc.vector.tensor_reduce(
                                    out=sc_sb[:, t : t + 1],
                                    in_=scr,
                                    axis=mybir.AxisListType.X,
                                    op=mybir.AluOpType.add,
                                )
                            else:
                                dump = dmpp.tile([128, ATT], BF16, tag="dmp")
                                nc.scalar.activation(
                                    dump,
                                    scr,
                                    Ident,
                                    accum_out=sc_sb[:, t : t + 1],
                                )
                return nat_b, sc_sb

            def emit_tail(b, nat_b, sc_sb):
                expf = smallp.tile([128, NT], F32, tag="expf")
                nc.scalar.activation(expf, sc_sb, Exp)
                nc.sync.dma_start(
                    out=att_d.ap()[b].rearrange("(t p) -> p t", p=128),
                    in_=expf,
                )
                expb = smallp.tile([128, NT], BF16, tag="expb")
                nc.vector.tensor_copy(expb, expf)
                cps = ctxps.tile([1, ENC], F32, tag="ctx")
                for t in range(NT):
                    nc.tensor.matmul(
                        cps,
                        lhsT=expb[:, t : t + 1],
                        rhs=nat_b[:, t, :],
                        start=(t == 0),
                        stop=(t == NT - 1),
                    )
                csb = smallp.tile([1, ENC], F32, tag="csb")
                nc.vector.tensor_copy(csb, cps)
                nc.sync.dma_start(out=ctx_d.ap()[b], in_=csb)

            pending = None
            for b in range(BPC):
                nat_b, sc = emit_scores(b)
                if pending is not None:
                    emit_tail(*pending)
                pending = (b, nat_b, sc)
            emit_tail(*pending)

    _split_sync_waits(nc)
    return nc


_NC = None
LAST_RESULT = None


def _get_nc():
    global _NC
    if _NC is None:
        _NC = build_nc()
    return _NC


def kernel(
    encoder_hidden_states,
    decoder_hidden_state,
    W_enc_w,
    W_enc_b,
    W_dec_w,
    W_dec_b,
    V_w,
    V_b,
    bias,
):
    from concourse.bass_utils import run_bass_kernel_spmd

    enc = np.asarray(encoder_hidden_states, dtype=np.float32)
    dec = np.asarray(decoder_hidden_state, dtype=np.float32)
    W_enc_w = np.asarray(W_enc_w, dtype=np.float32)
    W_enc_b = np.asarray(W_enc_b, dtype=np.float32)
    W_dec_w = np.asarray(W_dec_w, dtype=np.float32)
    W_dec_b = np.asarray(W_dec_b, dtype=np.float32)
    V_w = np.asarray(V_w, dtype=np.float32)
    bias = np.asarray(bias, dtype=np.float32)

    bf16 = ml_dtypes.bfloat16
    db = dec @ W_dec_w.T + W_dec_b + bias + W_enc_b  # [B, ATT]
    enc_bf = enc.astype(bf16)  # [B, S, ENC]
    # encT[b, c, j, p, k, s] = enc[b, (8c+k)*128 + s, 128j + p]
    encT_bf = np.ascontiguousarray(
        enc_bf.reshape(B, NCH, TPC, 128, 4, 128).transpose(0, 1, 4, 5, 2, 3)
    )
    wt_bf = np.ascontiguousarray(W_enc_w.T).astype(bf16)  # [ENC, ATT]
    db_bf = db.astype(bf16)
    vb_bf = np.broadcast_to(V_w[0], (128, ATT)).copy().astype(bf16)
    ones_bf = np.ones((1, 128), dtype=bf16)

    in_maps = []
    for i in range(N_CORES):
        sl = slice(BPC * i, BPC * (i + 1))
        in_maps.append(
            {
                "enc": enc_bf[sl],
                "encT": encT_bf[sl],
                "wt": wt_bf,
                "db": db_bf[sl][None],
                "vb": vb_bf,
                "ones": ones_bf,
            }
        )

    res = run_bass_kernel_spmd(_get_nc(), in_maps, core_ids=list(range(N_CORES)))
    global LAST_RESULT
    LAST_RESULT = res

    exp_s = np.concatenate(
        [res.results[i]["att"] for i in range(N_CORES)], axis=0
    )  # [B, S] unnormalized
    ctx_u = np.concatenate(
        [res.results[i]["ctx"] for i in range(N_CORES)], axis=0
    )  # [B, ENC] unnormalized
    d = exp_s.sum(axis=-1, keepdims=True)
    attn = (exp_s / d).astype(np.float32)
    context = (ctx_u / d).astype(np.float32)
    return context, attn


# revision 12
# speedup vs baseline: 1.1367x; 1.1367x over previous
"""Bahdanau attention TRN2 kernel (8 NeuronCores, data-parallel over batch).

Problem: B=32, S=4096, ENC=DEC=ATT=512.
  enc_score = enc @ W_enc^T + W_enc_b            [B,S,A]
  dec_score = dec @ W_dec^T + W_dec_b            [B,A]
  align  = tanh(enc_score + dec_score + bias)    [B,S,A]
  scores = align @ V + V_b                       [B,S]
  attn   = softmax(scores, -1)                   [B,S]
  context= attn @ enc                            [B,E]

Host-side prep (cheap, O(weights + one pass over enc)):
  - db = dec@W_dec^T + W_dec_b + bias + W_enc_b folds every per-(b,a)
    additive term into one [B,A] tensor. V_b is dropped entirely: softmax is
    shift invariant, so it cannot affect either output.
  - enc is shipped twice in bf16: natural layout (for the context matmuls)
    and pre-transposed 128x128 blocks (stationary operands for the score
    matmul) - no on-device transposes at all.

Device per example (4 per core):
  - score matmul: stationary = encT block [e=128, s=128], moving = W_encT
    chunk [e=128, a=512], PSUM f32 accumulation over 4 e-chunks; the db row
    is added with a K=1 ones-matmul into the same PSUM group.
  - tanh on ACT straight out of PSUM (FD=1024 per instruction).
  - scores: DVE multiply by V (bf16 2x mode), then free-dim reduce split
    between DVE tensor_reduce and ACT activation-accumulate.
  - exp on ACT ([128,32] per example; scores are bounded by sum|V| ~ 5.7 so
    no max-subtraction is needed). Unnormalized exp goes straight to DRAM.
  - context: 32 M=1 PE matmuls (exp_bf16 stationary, natural enc moving)
    accumulating [1,512] in PSUM.
  - Softmax normalization of both outputs happens on host, exactly.
"""

import sys

sys.path.insert(0, "/opt/trn_rl_repo")

import numpy as np
import ml_dtypes

import concourse.bass as bass
import concourse.mybir as mybir
from concourse.tile import TileContext

B, S, ENC, ATT = 32, 4096, 512, 512
N_CORES = 8
BPC = B // N_CORES  # examples per core
NT = S // 128       # s-tiles per example (32)
NCH = 4             # encT stream chunks per example
TPC = NT // NCH     # s-tiles per chunk (8)
BF16 = mybir.dt.bfloat16
F32 = mybir.dt.float32

# ---------------------------------------------------------------------------
# Workaround: this container's walrus accepts at most one sync-wait per
# instruction (two for EventSemaphore); Tile emits several. Split the extras
# onto single-wait NOPs on the same engine right before the instruction.
_PATCHED = False


def _patch_drain():
    global _PATCHED
    if _PATCHED:
        return
    from bass_rust import ScopedClock

    def _drain_and_barrier(self, tick_clock, wait_clock):
        probe = self.nc.sync.nop(nofuse=True, hint="drain_wait_hoist")
        wait_clock.add_sem_waits(
            probe.ins, ScopedClock({None: tick_clock.global_clock})
        )
        si = probe.ins.sync_info
        waits = list(si.on_wait or []) if si is not None else []
        if len(waits) > 1:
            si.on_wait = waits[:1]
            for w in waits[1:]:
                n2 = self.nc.sync.nop(nofuse=True, hint="drain_wait_hoist")
                n2.ins.sync_info = mybir.SyncInfo(on_wait=[w], on_update=[])
        self.nc.sync.drain()
        self.nc.all_engine_barrier()
        assert self.sems is not None
        popped = self.nc._tile_sem_poison_stack.pop()
        assert popped is self._sem_poison
        self.nc.clear_and_free_semaphores(list(self.sems.allocated().values()))
        self.nc.all_engine_barrier()

    TileContext._drain_and_barrier = _drain_and_barrier
    _PATCHED = True


def _split_sync_waits(nc):
    ctr = [0]

    def mknop(engine, wait):
        ctr[0] += 1
        n = mybir.InstNoOp(name=f"I-wsplit-{ctr[0]}", ins=[], outs=[])
        n.engine = engine
        n.sync_info = mybir.SyncInfo(on_wait=[wait], on_update=[])
        return n

    for fn in nc.m.functions:
        for bb in fn.blocks:
            out = []
            changed = False
            for inst in bb.instructions:
                si = inst.sync_info
                waits = list(si.on_wait) if (si and si.on_wait) else []
                cap = 2 if isinstance(inst, mybir.InstEventSemaphore) else 1
                if len(waits) > cap:
                    changed = True
                    for w in waits[: len(waits) - cap]:
                        out.append(mknop(inst.engine, w))
                    si.on_wait = waits[len(waits) - cap :]
                out.append(inst)
            if changed:
                bb.instructions = out


# ---------------------------------------------------------------------------
def build_nc():
    _patch_drain()
    nc = bass.Bass()
    enc_d = nc.dram_tensor("enc", [BPC, S, ENC], BF16, kind="ExternalInput")
    encT_d = nc.dram_tensor(
        "encT", [BPC, NCH, 4, 128, TPC, 128], BF16, kind="ExternalInput"
    )
    wt_d = nc.dram_tensor("wt", [ENC, ATT], BF16, kind="ExternalInput")
    db_d = nc.dram_tensor("db", [1, BPC, ATT], BF16, kind="ExternalInput")
    vb_d = nc.dram_tensor("vb", [128, ATT], BF16, kind="ExternalInput")
    ones_d = nc.dram_tensor("ones", [1, 128], BF16, kind="ExternalInput")
    ctx_d = nc.dram_tensor("ctx", [BPC, ENC], F32, kind="ExternalOutput")
    att_d = nc.dram_tensor("att", [BPC, S], F32, kind="ExternalOutput")

    Tanh = mybir.ActivationFunctionType.Tanh
    Exp = mybir.ActivationFunctionType.Exp
    Ident = mybir.ActivationFunctionType.Identity

    with TileContext(nc) as tc:
        import contextlib

        with contextlib.ExitStack() as ctx:
            consts = ctx.enter_context(tc.tile_pool(name="consts", bufs=1))
            natp = ctx.enter_context(tc.tile_pool(name="nat", bufs=1))
            etp = ctx.enter_context(tc.tile_pool(name="et", bufs=3))
            thp = ctx.enter_context(tc.tile_pool(name="th", bufs=3))
            scrp = ctx.enter_context(tc.tile_pool(name="scr", bufs=3))
            dmpp = ctx.enter_context(tc.tile_pool(name="dmp", bufs=3))
            scp = ctx.enter_context(tc.tile_pool(name="sc", bufs=1))
            smallp = ctx.enter_context(tc.tile_pool(name="small", bufs=2))
            mmps = ctx.enter_context(
                tc.tile_pool(name="mmps", bufs=2, space="PSUM")
            )
            ctxps = ctx.enter_context(
                tc.tile_pool(name="ctxps", bufs=2, space="PSUM")
            )

            wt_sb = consts.tile([128, 4, ATT], BF16)
            nc.sync.dma_start(
                out=wt_sb, in_=wt_d.ap().rearrange("(j p) a -> p j a", p=128)
            )
            vb_sb = consts.tile([128, ATT], BF16)
            nc.sync.dma_start(out=vb_sb, in_=vb_d.ap())
            db_sb = consts.tile([1, BPC, ATT], BF16)
            nc.sync.dma_start(out=db_sb, in_=db_d.ap())
            ones_sb = consts.tile([1, 128], BF16)
            nc.sync.dma_start(out=ones_sb, in_=ones_d.ap())

            def emit_scores(b):
                nat_b = natp.tile([128, NT, ENC], BF16, tag=f"nat{b % 2}")
                nc.sync.dma_start(
                    out=nat_b,
                    in_=enc_d.ap()[b].rearrange("(t p) e -> p t e", p=128),
                )
                sc_sb = scp.tile([128, NT], F32, tag=f"sc{b % 2}")
                for ch in range(NCH):
                    etc = etp.tile([128, 4, TPC, 128], BF16, tag="etc")
                    nc.sync.dma_start(
                        out=etc,
                        in_=encT_d.ap()[b][ch].rearrange(
                            "j p k s -> p j k s"
                        ),
                    )
                    for pair in range(TPC // 2):
                        ps = mmps.tile([128, 2, ATT], F32, tag="mm")
                        for k2 in range(2):
                            k = pair * 2 + k2
                            for j in range(4):
                                nc.tensor.matmul(
                                    ps[:, k2, :],
                                    lhsT=etc[:, j, k, :],
                                    rhs=wt_sb[:, j, :],
                                    start=(j == 0),
                                    stop=False,
                                )
                            nc.tensor.matmul(
                                ps[:, k2, :],
                                lhsT=ones_sb,
                                rhs=db_sb[:, b, :],
                                start=False,
                                stop=True,
                            )
                        th = thp.tile([128, 2, ATT], BF16, tag="th")
                        nc.scalar.activation(th, ps, Tanh)
                        for k2 in range(2):
                            t = ch * TPC + pair * 2 + k2
                            scr = scrp.tile([128, ATT], BF16, tag="scr")
                            nc.vector.tensor_mul(scr, th[:, k2, :], vb_sb)
                            if t % 3 != 2:
                                nc.vector.tensor_reduce(
                                    out=sc_sb[:, t : t + 1],
                                    in_=scr,
                                    axis=mybir.AxisListType.X,
                                    op=mybir.AluOpType.add,
                                )
                            else:
                                dump = dmpp.tile([128, ATT], BF16, tag="dmp")
                                nc.scalar.activation(
                                    dump,
                                    scr,
                                    Ident,
                                    accum_out=sc_sb[:, t : t + 1],
                                )
                return nat_b, sc_sb

            def emit_tail(b, nat_b, sc_sb):
                expf = smallp.tile([128, NT], F32, tag="expf")
                nc.scalar.activation(expf, sc_sb, Exp)
                nc.sync.dma_start(
                    out=att_d.ap()[b].rearrange("(t p) -> p t", p=128),
                    in_=expf,
                )
                expb = smallp.tile([128, NT], BF16, tag="expb")
                nc.vector.tensor_copy(expb, expf)
                cps = ctxps.tile([1, ENC], F32, tag="ctx")
                for t in range(NT):
                    nc.tensor.matmul(
                        cps,
                        lhsT=expb[:, t : t + 1],
                        rhs=nat_b[:, t, :],
                        start=(t == 0),
                        stop=(t == NT - 1),
                    )
                csb = smallp.tile([1, ENC], F32, tag="csb")
                nc.vector.tensor_copy(csb, cps)
                nc.sync.dma_start(out=ctx_d.ap()[b], in_=csb)

            pending = None
            for b in range(BPC):
                nat_b, sc = emit_scores(b)
                if pending is not None:
                    emit_tail(*pending)
                pending = (b, nat_b, sc)
            emit_tail(*pending)

    _split_sync_waits(nc)
    return nc


_NC = None
LAST_RESULT = None


def _get_nc():
    global _NC
    if _NC is None:
        _NC = build_nc()
    return _NC


def kernel(
    encoder_hidden_states,
    decoder_hidden_state,
    W_enc_w,
    W_enc_b,
    W_dec_w,
    W_dec_b,
    V_w,
    V_b,
    bias,
):
    from concourse.bass_utils import run_bass_kernel_spmd

    enc = np.asarray(encoder_hidden_states, dtype=np.float32)
    dec = np.asarray(decoder_hidden_state, dtype=np.float32)
    W_enc_w = np.asarray(W_enc_w, dtype=np.float32)
    W_enc_b = np.asarray(W_enc_b, dtype=np.float32)
    W_dec_w = np.asarray(W_dec_w, dtype=np.float32)
    W_dec_b = np.asarray(W_dec_b, dtype=np.float32)
    V_w = np.asarray(V_w, dtype=np.float32)
    bias = np.asarray(bias, dtype=np.float32)

    bf16 = ml_dtypes.bfloat16
    db = dec @ W_dec_w.T + W_dec_b + bias + W_enc_b  # [B, ATT]
    enc_bf = enc.astype(bf16)  # [B, S, ENC]
    # encT[b, c, j, p, k, s] = enc[b, (8c+k)*128 + s, 128j + p]
    encT_bf = np.ascontiguousarray(
        enc_bf.reshape(B, NCH, TPC, 128, 4, 128).transpose(0, 1, 4, 5, 2, 3)
    )
    wt_bf = np.ascontiguousarray(W_enc_w.T).astype(bf16)  # [ENC, ATT]
    db_bf = db.astype(bf16)
    vb_bf = np.broadcast_to(V_w[0], (128, ATT)).copy().astype(bf16)
    ones_bf = np.ones((1, 128), dtype=bf16)

    in_maps = []
    for i in range(N_CORES):
        sl = slice(BPC * i, BPC * (i + 1))
        in_maps.append(
            {
                "enc": enc_bf[sl],
                "encT": encT_bf[sl],
                "wt": wt_bf,
                "db": db_bf[sl][None],
                "vb": vb_bf,
                "ones": ones_bf,
            }
        )

    res = run_bass_kernel_spmd(_get_nc(), in_maps, core_ids=list(range(N_CORES)))
    global LAST_RESULT
    LAST_RESULT = res

    exp_s = np.concatenate(
        [res.results[i]["att"] for i in range(N_CORES)], axis=0
    )  # [B, S] unnormalized
    ctx_u = np.concatenate(
        [res.results[i]["ctx"] for i in range(N_CORES)], axis=0
    )  # [B, ENC] unnormalized
    d = exp_s.sum(axis=-1, keepdims=True)
    attn = (exp_s / d).astype(np.float32)
    context = (ctx_u / d).astype(np.float32)
    return context, attn


# revision 13
# speedup vs baseline: 1.1512x; 1.0128x over previous
"""Bahdanau attention TRN2 kernel (8 NeuronCores, data-parallel over batch).

Problem: B=32, S=4096, ENC=DEC=ATT=512.
  enc_score = enc @ W_enc^T + W_enc_b            [B,S,A]
  dec_score = dec @ W_dec^T + W_dec_b            [B,A]
  align  = tanh(enc_score + dec_score + bias)    [B,S,A]
  scores = align @ V + V_b                       [B,S]
  attn   = softmax(scores, -1)                   [B,S]
  context= attn @ enc                            [B,E]

Host-side prep (cheap, O(weights + one pass over enc)):
  - db = dec@W_dec^T + W_dec_b + bias + W_enc_b folds every per-(b,a)
    additive term into one [B,A] tensor. V_b is dropped entirely: softmax is
    shift invariant, so it cannot affect either output.
  - enc is shipped twice in bf16: natural layout (for the context matmuls)
    and pre-transposed 128x128 blocks (stationary operands for the score
    matmul) - no on-device transposes at all.

Device per example (4 per core):
  - score matmul: stationary = encT block [e=128, s=128], moving = W_encT
    chunk [e=128, a=512], PSUM f32 accumulation over 4 e-chunks; the db row
    is added with a K=1 ones-matmul into the same PSUM group.
  - tanh on ACT straight out of PSUM (FD=1024 per instruction).
  - scores: DVE multiply by V (bf16 2x mode), then free-dim reduce split
    between DVE tensor_reduce and ACT activation-accumulate.
  - exp on ACT ([128,32] per example; scores are bounded by sum|V| ~ 5.7 so
    no max-subtraction is needed). Unnormalized exp goes straight to DRAM.
  - context: 32 M=1 PE matmuls (exp_bf16 stationary, natural enc moving)
    accumulating [1,512] in PSUM.
  - Softmax normalization of both outputs happens on host, exactly.
"""

import sys

sys.path.insert(0, "/opt/trn_rl_repo")

import numpy as np
import ml_dtypes

import concourse.bass as bass
import concourse.mybir as mybir
from concourse.tile import TileContext

B, S, ENC, ATT = 32, 4096, 512, 512
N_CORES = 8
BPC = B // N_CORES  # examples per core
NT = S // 128       # s-tiles per example (32)
NCH = 4             # encT stream chunks per example
TPC = NT // NCH     # s-tiles per chunk (8)
BF16 = mybir.dt.bfloat16
F32 = mybir.dt.float32

# ---------------------------------------------------------------------------
# Workaround: this container's walrus accepts at most one sync-wait per
# instruction (two for EventSemaphore); Tile emits several. Split the extras
# onto single-wait NOPs on the same engine right before the instruction.
_PATCHED = False


def _patch_drain():
    global _PATCHED
    if _PATCHED:
        return
    from bass_rust import ScopedClock

    def _drain_and_barrier(self, tick_clock, wait_clock):
        probe = self.nc.sync.nop(nofuse=True, hint="drain_wait_hoist")
        wait_clock.add_sem_waits(
            probe.ins, ScopedClock({None: tick_clock.global_clock})
        )
        si = probe.ins.sync_info
        waits = list(si.on_wait or []) if si is not None else []
        if len(waits) > 1:
            si.on_wait = waits[:1]
            for w in waits[1:]:
                n2 = self.nc.sync.nop(nofuse=True, hint="drain_wait_hoist")
                n2.ins.sync_info = mybir.SyncInfo(on_wait=[w], on_update=[])
        self.nc.sync.drain()
        self.nc.all_engine_barrier()
        assert self.sems is not None
        popped = self.nc._tile_sem_poison_stack.pop()
        assert popped is self._sem_poison
        self.nc.clear_and_free_semaphores(list(self.sems.allocated().values()))
        self.nc.all_engine_barrier()

    TileContext._drain_and_barrier = _drain_and_barrier
    _PATCHED = True


def _split_sync_waits(nc):
    ctr = [0]

    def mknop(engine, wait):
        ctr[0] += 1
        n = mybir.InstNoOp(name=f"I-wsplit-{ctr[0]}", ins=[], outs=[])
        n.engine = engine
        n.sync_info = mybir.SyncInfo(on_wait=[wait], on_update=[])
        return n

    for fn in nc.m.functions:
        for bb in fn.blocks:
            out = []
            changed = False
            for inst in bb.instructions:
                si = inst.sync_info
                waits = list(si.on_wait) if (si and si.on_wait) else []
                cap = 2 if isinstance(inst, mybir.InstEventSemaphore) else 1
                if len(waits) > cap:
                    changed = True
                    for w in waits[: len(waits) - cap]:
                        out.append(mknop(inst.engine, w))
                    si.on_wait = waits[len(waits) - cap :]
                out.append(inst)
            if changed:
                bb.instructions = out


# ---------------------------------------------------------------------------
def build_nc():
    _patch_drain()
    nc = bass.Bass()
    enc_d = nc.dram_tensor("enc", [BPC, S, ENC], BF16, kind="ExternalInput")
    encT_d = nc.dram_tensor(
        "encT", [BPC, NCH, 4, 128, TPC, 128], BF16, kind="ExternalInput"
    )
    wt_d = nc.dram_tensor("wt", [ENC, ATT], BF16, kind="ExternalInput")
    db_d = nc.dram_tensor("db", [1, BPC, ATT], BF16, kind="ExternalInput")
    vb_d = nc.dram_tensor("vb", [128, ATT], BF16, kind="ExternalInput")
    ones_d = nc.dram_tensor("ones", [1, 128], BF16, kind="ExternalInput")
    ctx_d = nc.dram_tensor("ctx", [BPC, ENC], F32, kind="ExternalOutput")
    att_d = nc.dram_tensor("att", [BPC, S], F32, kind="ExternalOutput")

    Tanh = mybir.ActivationFunctionType.Tanh
    Exp = mybir.ActivationFunctionType.Exp
    Ident = mybir.ActivationFunctionType.Identity

    with TileContext(nc) as tc:
        import contextlib

        with contextlib.ExitStack() as ctx:
            consts = ctx.enter_context(tc.tile_pool(name="consts", bufs=1))
            natp = ctx.enter_context(tc.tile_pool(name="nat", bufs=1))
            etp = ctx.enter_context(tc.tile_pool(name="et", bufs=3))
            thp = ctx.enter_context(tc.tile_pool(name="th", bufs=3))
            scrp = ctx.enter_context(tc.tile_pool(name="scr", bufs=3))
            dmpp = ctx.enter_context(tc.tile_pool(name="dmp", bufs=3))
            scp = ctx.enter_context(tc.tile_pool(name="sc", bufs=1))
            smallp = ctx.enter_context(tc.tile_pool(name="small", bufs=2))
            mmps = ctx.enter_context(
                tc.tile_pool(name="mmps", bufs=2, space="PSUM")
            )
            ctxps = ctx.enter_context(
                tc.tile_pool(name="ctxps", bufs=2, space="PSUM")
            )

            wt_sb = consts.tile([128, 4, ATT], BF16)
            nc.sync.dma_start(
                out=wt_sb, in_=wt_d.ap().rearrange("(j p) a -> p j a", p=128)
            )
            vb_sb = consts.tile([128, ATT], BF16)
            nc.sync.dma_start(out=vb_sb, in_=vb_d.ap())
            db_sb = consts.tile([1, BPC, ATT], BF16)
            nc.sync.dma_start(out=db_sb, in_=db_d.ap())
            ones_sb = consts.tile([1, 128], BF16)
            nc.sync.dma_start(out=ones_sb, in_=ones_d.ap())

            def emit_scores(b):
                # natural-layout enc rides the SWDGE queue so it never
                # head-of-line blocks the encT chunks the PE is waiting on
                nat_b = natp.tile([128, NT, ENC], BF16, tag=f"nat{b % 2}")
                nc.gpsimd.dma_start(
                    out=nat_b,
                    in_=enc_d.ap()[b].rearrange("(t p) e -> p t e", p=128),
                )
                sc_sb = scp.tile([128, NT], F32, tag=f"sc{b % 2}")
                for ch in range(NCH):
                    etc = etp.tile([128, 4, TPC, 128], BF16, tag="etc")
                    nc.sync.dma_start(
                        out=etc,
                        in_=encT_d.ap()[b][ch].rearrange(
                            "j p k s -> p j k s"
                        ),
                    )
                    for pair in range(TPC // 2):
                        ps = mmps.tile([128, 2, ATT], F32, tag="mm")
                        for k2 in range(2):
                            k = pair * 2 + k2
                            for j in range(4):
                                nc.tensor.matmul(
                                    ps[:, k2, :],
                                    lhsT=etc[:, j, k, :],
                                    rhs=wt_sb[:, j, :],
                                    start=(j == 0),
                                    stop=False,
                                )
                            nc.tensor.matmul(
                                ps[:, k2, :],
                                lhsT=ones_sb,
                                rhs=db_sb[:, b, :],
                                start=False,
                                stop=True,
                            )
                        th = thp.tile([128, 2, ATT], BF16, tag="th")
                        nc.scalar.activation(th, ps, Tanh)
                        for k2 in range(2):
                            t = ch * TPC + pair * 2 + k2
                            scr = scrp.tile([128, ATT], BF16, tag="scr")
                            nc.vector.tensor_mul(scr, th[:, k2, :], vb_sb)
                            if t % 3 != 2:
                                nc.vector.tensor_reduce(
                                    out=sc_sb[:, t : t + 1],
                                    in_=scr,
                                    axis=mybir.AxisListType.X,
                                    op=mybir.AluOpType.add,
                                )
                            else:
                                dump = dmpp.tile([128, ATT], BF16, tag="dmp")
                                nc.scalar.activation(
                                    dump,
                                    scr,
                                    Ident,
                                    accum_out=sc_sb[:, t : t + 1],
                                )
                return nat_b, sc_sb

            def emit_tail(b, nat_b, sc_sb):
                expf = smallp.tile([128, NT], F32, tag="expf")
                nc.scalar.activation(expf, sc_sb, Exp)
                nc.sync.dma_start(
                    out=att_d.ap()[b].rearrange("(t p) -> p t", p=128),
                    in_=expf,
                )
                expb = smallp.tile([128, NT], BF16, tag="expb")
                nc.vector.tensor_copy(expb, expf)
                cps = ctxps.tile([1, ENC], F32, tag="ctx")
                for t in range(NT):
                    nc.tensor.matmul(
                        cps,
                        lhsT=expb[:, t : t + 1],
                        rhs=nat_b[:, t, :],
                        start=(t == 0),
                        stop=(t == NT - 1),
                    )
                csb = smallp.tile([1, ENC], F32, tag="csb")
                nc.vector.tensor_copy(csb, cps)
                nc.sync.dma_start(out=ctx_d.ap()[b], in_=csb)

            pending = None
            for b in range(BPC):
                nat_b, sc = emit_scores(b)
                if pending is not None:
                    emit_tail(*pending)
                pending = (b, nat_b, sc)
            emit_tail(*pending)

    _split_sync_waits(nc)
    return nc


_NC = None
LAST_RESULT = None


def _get_nc():
    global _NC
    if _NC is None:
        _NC = build_nc()
    return _NC


def kernel(
    encoder_hidden_states,
    decoder_hidden_state,
    W_enc_w,
    W_enc_b,
    W_dec_w,
    W_dec_b,
    V_w,
    V_b,
    bias,
):
    from concourse.bass_utils import run_bass_kernel_spmd

    enc = np.asarray(encoder_hidden_states, dtype=np.float32)
    dec = np.asarray(decoder_hidden_state, dtype=np.float32)
    W_enc_w = np.asarray(W_enc_w, dtype=np.float32)
    W_enc_b = np.asarray(W_enc_b, dtype=np.float32)
    W_dec_w = np.asarray(W_dec_w, dtype=np.float32)
    W_dec_b = np.asarray(W_dec_b, dtype=np.float32)
    V_w = np.asarray(V_w, dtype=np.float32)
    bias = np.asarray(bias, dtype=np.float32)

    bf16 = ml_dtypes.bfloat16
    db = dec @ W_dec_w.T + W_dec_b + bias + W_enc_b  # [B, ATT]
    enc_bf = enc.astype(bf16)  # [B, S, ENC]
    # encT[b, c, j, p, k, s] = enc[b, (8c+k)*128 + s, 128j + p]
    encT_bf = np.ascontiguousarray(
        enc_bf.reshape(B, NCH, TPC, 128, 4, 128).transpose(0, 1, 4, 5, 2, 3)
    )
    wt_bf = np.ascontiguousarray(W_enc_w.T).astype(bf16)  # [ENC, ATT]
    db_bf = db.astype(bf16)
    vb_bf = np.broadcast_to(V_w[0], (128, ATT)).copy().astype(bf16)
    ones_bf = np.ones((1, 128), dtype=bf16)

    in_maps = []
    for i in range(N_CORES):
        sl = slice(BPC * i, BPC * (i + 1))
        in_maps.append(
            {
                "enc": enc_bf[sl],
                "encT": encT_bf[sl],
                "wt": wt_bf,
                "db": db_bf[sl][None],
                "vb": vb_bf,
                "ones": ones_bf,
            }
        )

    res = run_bass_kernel_spmd(_get_nc(), in_maps, core_ids=list(range(N_CORES)))
    global LAST_RESULT
    LAST_RESULT = res

    exp_s = np.concatenate(
        [res.results[i]["att"] for i in range(N_CORES)], axis=0
    )  # [B, S] unnormalized
    ctx_u = np.concatenate(
        [res.results[i]["ctx"] for i in range(N_CORES)], axis=0
    )  # [B, ENC] unnormalized
    d = exp_s.sum(axis=-1, keepdims=True)
    attn = (exp_s / d).astype(np.float32)
    context = (ctx_u / d).astype(np.float32)
    return context, attn
